# revision 8
# baseline (speedup 1.0000x reference)
"""Multi-head causal attention (B=2, S=2048, E=1024, H=16, D=64) on 8 TRN2
NeuronCores.

Sharding (data + tensor parallel, Megatron-style):
  core c -> batch b = c // 4, head group g = c % 4 (4 heads, e' = 256 cols).
  Wq/Wk/Wv column-sharded ([256, 1024] slices), Wo row-sharded
  ([1024, 256] slice); each core produces a partial output [2048, 1024]
  (f16) which the host sums per batch group (the Megatron all-reduce) and
  adds bo.

Per-core device kernel (matmul operands fp16, accumulate fp32 in PSUM),
structured so the ACT(exp) stream starts ~15us in and never stalls:
  - x tensors arrive host-pre-blocked as [4, 128, 8*512] so each 512-col
    block is one 128-descriptor DMA; blocks are ordered by first use
    (xk asc, xq desc, xv asc) and the K/Q projections chase the DMAs.
  - attention in S^T orientation per (q-tile 512, head-pair chunk c):
    batches of 8 k-tiles: logits pairs (2 heads row-tiled on the PE at
    partitions 0-63/64-127), exp on ACT (1/8 scale folded), triangular
    trimming (diagonal tiles only compute columns >= dd, one [128,2,128]
    lower-tri mask), attnV (V' ones column -> PSUM row 64 = softmax
    denominator) lagging the exp stream by 2 tiles.
  - leftover K/Q projection tiles, V-projection tiles and the previous
    q-tile's O-projection are woven into the attention stream at batch
    boundaries (all 128-contraction) to use the PE slack under ACT.
  - normalize: accs copied to SBUF (frees PSUM fast), denominator rows
    packed [2,512], reciprocal via bit-trick seed + 2 Newton iterations
    on the idle GpSimd(Pool) engine, partition-broadcast, DVE multiply
    into valsT.
"""
import sys
import os

sys.path.insert(0, "/opt/trn_rl_repo")

import numpy as np
from contextlib import ExitStack

import concourse.bass as bass  # noqa: E402
import concourse.mybir as mybir  # noqa: E402
import concourse.tile as tile  # noqa: E402
from concourse import bacc, bass_utils  # noqa: E402

bass_utils.upload_artifacts = lambda d: f"local:{d}"

B, S, E, H, D = 2, 2048, 1024, 16, 64
NCORES = 8
EL = 256  # e' columns per core (4 heads)
F32 = mybir.dt.float32
F16 = mybir.dt.float16
I32 = mybir.dt.int32
AF = mybir.ActivationFunctionType
OP = mybir.AluOpType
NP16 = np.float16

RECIP_SEED = 0x7EF311C3 + 1  # ~(x) + (C+1) == C - x

_CACHE = {}


def _build():
    nc = bacc.Bacc("TRN2", target_bir_lowering=False, debug=False)

    # x tensors host-pre-blocked: [tb, p, k*512 + m]
    xq_d = nc.dram_tensor("xqB", [4, 128, 8 * 512], F16, kind="ExternalInput")
    xk_d = nc.dram_tensor("xkB", [4, 128, 8 * 512], F16, kind="ExternalInput")
    xv_d = nc.dram_tensor("xvB", [4, 128, 8 * 512], F16, kind="ExternalInput")
    wq_d = nc.dram_tensor("wqT", [E, EL], F16, kind="ExternalInput")
    wk_d = nc.dram_tensor("wkT", [E, EL], F16, kind="ExternalInput")
    wv_d = nc.dram_tensor("wvT", [E, EL], F16, kind="ExternalInput")
    wo_d = nc.dram_tensor("woT", [EL, E], F16, kind="ExternalInput")
    bq_d = nc.dram_tensor("bq", [EL], F32, kind="ExternalInput")
    bk_d = nc.dram_tensor("bk", [EL], F32, kind="ExternalInput")
    bv_d = nc.dram_tensor("bv", [EL], F32, kind="ExternalInput")
    vones_d = nc.dram_tensor("vones", [128, 16, 4, 1], F16, kind="ExternalInput")
    mask_d = nc.dram_tensor("masks", [128, 2, 128], F16, kind="ExternalInput")
    out_d = nc.dram_tensor("out", [S, E], F16, kind="ExternalOutput")

    with tile.TileContext(nc) as tc, ExitStack() as ctx:
        cpool = ctx.enter_context(tc.tile_pool(name="const", bufs=1))
        psp = ctx.enter_context(tc.tile_pool(name="psp", bufs=2, space="PSUM"))
        expp = ctx.enter_context(tc.tile_pool(name="expp", bufs=10))
        opool = ctx.enter_context(tc.tile_pool(name="op", bufs=2))
        smp = ctx.enter_context(tc.tile_pool(name="smp", bufs=2))

        xk = cpool.tile([128, 8, S], F16, tag="xk")
        xq = cpool.tile([128, 8, S], F16, tag="xq")
        xv = cpool.tile([128, 8, S], F16, tag="xv")

        def xblock(x_t, x_d, tb):
            nc.sync.dma_start(
                x_t[:, :, tb * 512:(tb + 1) * 512],
                x_d.ap()[tb].rearrange("p (k m) -> p k m", k=8))

        # ---- DMA order == arrival order (single HWDGE ring): weights for
        # K/Q, xk/xq/xv blocks interleaved by first-use time ----
        wk = cpool.tile([128, 8, EL], F16, tag="wk")
        nc.sync.dma_start(wk[:], wk_d.ap().rearrange("(k p) m -> p k m", p=128))
        bkt = cpool.tile([128, 2], F32, tag="bkt")
        nc.sync.dma_start(bkt[:], bk_d.ap().rearrange("(c p) -> p c", p=128))
        wq = cpool.tile([128, 8, EL], F16, tag="wq")
        nc.sync.dma_start(wq[:], wq_d.ap().rearrange("(k p) m -> p k m", p=128))
        bqt = cpool.tile([128, 2], F32, tag="bqt")
        nc.sync.dma_start(bqt[:], bq_d.ap().rearrange("(c p) -> p c", p=128))
        xblock(xk, xk_d, 0)
        xblock(xk, xk_d, 1)
        xblock(xq, xq_d, 3)
        xblock(xk, xk_d, 2)
        xblock(xk, xk_d, 3)
        xblock(xq, xq_d, 2)
        wv = cpool.tile([128, 8, EL], F16, tag="wv")
        nc.sync.dma_start(wv[:], wv_d.ap().rearrange("(k p) m -> p k m", p=128))
        bvr = cpool.tile([1, EL], F32, tag="bvr")
        nc.sync.dma_start(bvr[:], bv_d.ap().rearrange("(p m) -> p m", p=1))
        bvb = cpool.tile([128, EL], F32, tag="bvb")
        nc.gpsimd.partition_broadcast(bvb[:], bvr[:])
        VP = cpool.tile([128, 16, 4 * 66], F16, tag="VP")  # 66: 4B-aligned
        nc.sync.dma_start(
            VP[:].rearrange("p k (h x) -> p k h x", h=4)[:, :, :, 64:65],
            vones_d.ap())
        xblock(xv, xv_d, 0)
        xblock(xv, xv_d, 1)
        xblock(xq, xq_d, 1)
        xblock(xq, xq_d, 0)
        xblock(xv, xv_d, 2)
        xblock(xv, xv_d, 3)
        mk2 = cpool.tile([128, 2, 128], F16, tag="mk2")
        nc.sync.dma_start(mk2[:], mask_d.ap())
        wo = cpool.tile([128, 2, E], F16, tag="wo")
        nc.sync.dma_start(wo[:], wo_d.ap().rearrange("(c p) m -> p c m", p=128))

        KT = cpool.tile([128, 2, S], F16, tag="KT")
        QT = cpool.tile([128, 2, S], F16, tag="QT")
        valsT = cpool.tile([128, 2, S], F16, tag="valsT")

        def kproj_tb(tb, tag):
            for c in range(2):
                ps = psp.tile([128, 512], F32, tag=tag,
                              bufs=2, name=f"kps{tb}_{c}")
                for k in range(8):
                    nc.tensor.matmul(
                        ps[:],
                        lhsT=wk[:, k, c * 128:(c + 1) * 128],
                        rhs=xk[:, k, tb * 512:(tb + 1) * 512],
                        start=(k == 0), stop=(k == 7))
                nc.vector.tensor_scalar_add(
                    KT[:, c, tb * 512:(tb + 1) * 512], ps[:], bkt[:, c:c + 1])

        def qproj_tt(tt, tag):
            for c in range(2):
                ps = psp.tile([128, 512], F32, tag=tag,
                              bufs=2, name=f"qps{tt}_{c}")
                for k in range(8):
                    nc.tensor.matmul(
                        ps[:],
                        lhsT=wq[:, k, c * 128:(c + 1) * 128],
                        rhs=xq[:, k, tt * 512:(tt + 1) * 512],
                        start=(k == 0), stop=(k == 7))
                nc.vector.tensor_scalar_add(
                    QT[:, c, tt * 512:(tt + 1) * 512], ps[:], bqt[:, c:c + 1])

        # K blocks 0,1 + Q tile 3 up front: first exp fires after these
        kproj_tb(0, "lg")
        kproj_tb(1, "lg")
        qproj_tt(3, "lg")

        def vproj_tile(t3):
            ps = psp.tile([128, EL], F32, tag="ops", bufs=2, name=f"vps{t3}")
            for k in range(8):
                nc.tensor.matmul(
                    ps[:],
                    lhsT=xv[:, k, t3 * 128:(t3 + 1) * 128],
                    rhs=wv[:, k, :],
                    start=(k == 0), stop=(k == 7))
            nc.vector.tensor_add(
                VP[:, t3, :].rearrange("p (h x) -> p h x", h=4)[:, :, 0:64],
                ps[:].rearrange("p (h x) -> p h x", h=4),
                bvb[:].rearrange("p (h x) -> p h x", h=4))

        def oproj_tt(tt):
            ot = opool.tile([128, 2, 512], F16, tag="ot", name=f"ot{tt}")
            for eo in range(2):
                ps = psp.tile([128, 512], F32, tag="ops", bufs=2,
                              name=f"ops{tt}_{eo}")
                for c in range(2):
                    nc.tensor.matmul(
                        ps[:],
                        lhsT=valsT[:, c, tt * 128:(tt + 1) * 128],
                        rhs=wo[:, c, eo * 512:(eo + 1) * 512],
                        start=(c == 0), stop=(c == 1))
                nc.vector.tensor_copy(ot[:, eo, :], ps[:])
            nc.sync.dma_start(
                out_d.ap()[tt * 128:(tt + 1) * 128, :],
                ot[:].rearrange("p a b -> p (a b)"))

        # 128-contraction work woven in at attention batch boundaries
        boundary = {
            (3, 0, 0): lambda: (kproj_tb(2, "ops"), kproj_tb(3, "ops")),
            (3, 0, 8): lambda: qproj_tt(2, "ops"),
            (3, 1, 0): lambda: qproj_tt(1, "ops"),
            (3, 1, 8): lambda: qproj_tt(0, "ops"),
            (2, 0, 0): lambda: (oproj_tt(12), oproj_tt(13)),
            (2, 0, 8): lambda: (oproj_tt(14), oproj_tt(15)),
            (1, 0, 0): lambda: (oproj_tt(8), oproj_tt(9)),
            (1, 1, 0): lambda: (oproj_tt(10), oproj_tt(11)),
            (0, 0, 0): lambda: (oproj_tt(4), oproj_tt(5)),
            (0, 1, 0): lambda: (oproj_tt(6), oproj_tt(7)),
        }

        for qt in range(3, -1, -1):
            nkt = 4 * qt + 4
            accs = {}
            for c in range(2):
                for hh in range(2):
                    accs[(c, hh)] = psp.tile([65, 512], F32, tag="acc",
                                             bufs=2, name=f"acc{qt}_{c}_{hh}")
            exs = {}

            def lg_exp(c, kt):
                dd = kt * 128 - qt * 512
                s = max(dd, 0)
                lg = psp.tile([128, 2, 512], F32, tag="lg", bufs=2,
                              name=f"lg{qt}_{c}_{kt}")
                for hh in range(2):
                    nc.tensor.matmul(
                        lg[:, hh, s:512],
                        lhsT=KT[hh * 64:(hh + 1) * 64, c,
                                kt * 128:(kt + 1) * 128],
                        rhs=QT[hh * 64:(hh + 1) * 64, c,
                               qt * 512 + s:(qt + 1) * 512],
                        start=True, stop=True)
                ex = expp.tile([128, 2, 512], F16, tag="ex",
                               name=f"ex{qt}_{c}_{kt}")
                nc.scalar.activation(ex[:, :, s:512], lg[:, :, s:512], AF.Exp,
                                     scale=0.125)
                if dd >= 0:  # diagonal tile: lower-tri mask on first 128 cols
                    nc.vector.tensor_mul(ex[:, :, s:s + 128],
                                         ex[:, :, s:s + 128], mk2[:])
                exs[(c, kt)] = ex

            def attn_v(c, kt):
                ex = exs.pop((c, kt))
                s = max(kt * 128 - qt * 512, 0)
                for hh in range(2):
                    h = 2 * c + hh
                    nc.tensor.matmul(
                        accs[(c, hh)][:, s:512],
                        lhsT=VP[:, kt, h * 66:h * 66 + 65],
                        rhs=ex[:, hh, s:512],
                        start=(kt == 0), stop=(kt == nkt - 1),
                        skip_group_check=True)

            for c in range(2):
                pend = 0
                for b0 in range(0, nkt, 8):
                    bend = min(b0 + 8, nkt)
                    for kt in range(b0, bend):
                        lg_exp(c, kt)
                    bw = boundary.get((qt, c, b0))
                    if bw is not None:
                        bw()
                    while pend <= bend - 3:
                        if qt == 3 and c == 0:
                            vproj_tile(pend)
                        attn_v(c, pend)
                        pend += 1
                while pend < nkt:
                    if qt == 3 and c == 0:
                        vproj_tile(pend)
                    attn_v(c, pend)
                    pend += 1

                # ---- normalize chunk c ----
                # denominator rows (PSUM row 64) -> srow2 rows 0/1; acc body
                # -> SBUF sv (frees the PSUM slot); reciprocal via bit-trick
                # seed + 2 Newton iterations on Pool; broadcast; multiply.
                sr2 = smp.tile([1, 2, 512], F32, tag="sr2", bufs=1,
                               name=f"sr2_{qt}_{c}")
                svs = {}
                for hh in range(2):
                    nc.vector.tensor_copy(sr2[0:1, hh, :],
                                          accs[(c, hh)][64:65, :])
                    sv = smp.tile([65, 512], F32, tag="sv", bufs=4,
                                  name=f"sv{qt}_{c}_{hh}")
                    nc.vector.tensor_copy(sv[:], accs[(c, hh)][:])
                    svs[hh] = sv
                rc = smp.tile([1, 2, 512], F32, tag="rc", bufs=1,
                               name=f"rc{qt}_{c}")
                tm = smp.tile([1, 2, 512], F32, tag="tm", bufs=1,
                               name=f"tm{qt}_{c}")
                nc.gpsimd.tensor_scalar(
                    rc[:].bitcast(I32), sr2[:].bitcast(I32),
                    -1, RECIP_SEED - 1, OP.mult, OP.add)
                for _ in range(2):  # Newton: r = r * (2 - x * r)
                    nc.gpsimd.tensor_tensor(tm[:], sr2[:], rc[:], OP.mult)
                    nc.gpsimd.tensor_scalar(tm[:], tm[:],
                                            -1.0, 2.0, OP.mult, OP.add)
                    nc.gpsimd.tensor_tensor(rc[:], rc[:], tm[:], OP.mult)
                for hh in range(2):
                    bc = smp.tile([128, 512], F32, tag="bc", bufs=4,
                                  name=f"bc{qt}_{c}_{hh}")
                    nc.gpsimd.partition_broadcast(bc[0:64, :], rc[0:1, hh, :])
                    nc.vector.tensor_mul(
                        valsT[hh * 64:(hh + 1) * 64, c,
                              qt * 512:(qt + 1) * 512],
                        svs[hh][0:64, :], bc[0:64, :])

        # tail: first q-tile's O-projection
        for tt in range(4):
            oproj_tt(tt)

    nc.compile()
    return nc


def get_nc():
    if "nc" not in _CACHE:
        _CACHE["nc"] = _build()
    return _CACHE["nc"]


def _masks():
    i = np.arange(128)[:, None]
    j = np.arange(128)[None, :]
    m = (i <= j).astype(NP16)  # within-window causal: keep k <= q
    return np.broadcast_to(m[:, None, :], (128, 2, 128)).copy()


def _xblocks(x):
    # [S, E] f32 -> [4, 128, 8*512] f16: blk[tb, p, k*512+m] = x[tb*512+m, k*128+p]
    xT = np.ascontiguousarray(x.T).astype(NP16)  # [E, S]
    return np.ascontiguousarray(
        xT.reshape(8, 128, 4, 512).transpose(2, 1, 0, 3).reshape(4, 128, 4096))


def make_in_maps(query, key, value, Wq, bq, Wk, bk, Wv, bv, Wo, bo):
    query = np.asarray(query, np.float32)
    key = np.asarray(key, np.float32)
    value = np.asarray(value, np.float32)
    Wq, Wk, Wv, Wo = (np.asarray(a, np.float32) for a in (Wq, Wk, Wv, Wo))
    bq, bk, bv = (np.asarray(a, np.float32) for a in (bq, bk, bv))
    masks = _masks()
    vones = np.ones((128, 16, 4, 1), NP16)
    xb = {}
    for b in range(B):
        xb[b] = (_xblocks(query[b]), _xblocks(key[b]), _xblocks(value[b]))
    in_maps = []
    for c in range(NCORES):
        b, g = divmod(c, 4)
        sl = slice(g * EL, (g + 1) * EL)
        in_maps.append({
            "xqB": xb[b][0],
            "xkB": xb[b][1],
            "xvB": xb[b][2],
            "wqT": np.ascontiguousarray(Wq[sl, :].T).astype(NP16),
            "wkT": np.ascontiguousarray(Wk[sl, :].T).astype(NP16),
            "wvT": np.ascontiguousarray(Wv[sl, :].T).astype(NP16),
            "woT": np.ascontiguousarray(Wo[:, sl].T).astype(NP16),
            "bq": np.ascontiguousarray(bq[sl]),
            "bk": np.ascontiguousarray(bk[sl]),
            "bv": np.ascontiguousarray(bv[sl]),
            "vones": vones,
            "masks": masks,
        })
    return in_maps


def run(inputs, trace=False, tmpdir=None):
    """Run on 8 cores; returns (full_output, BassKernelResults)."""
    nc = get_nc()
    in_maps = make_in_maps(**inputs)
    res = bass_utils.run_bass_kernel_spmd(
        nc, in_maps, list(range(NCORES)), trace=trace, tmpdir=tmpdir)
    bo = np.asarray(inputs["bo"], np.float32)
    out = np.zeros((B, S, E), np.float32)
    for c in range(NCORES):
        out[c // 4] += res.results[c]["out"]
    out += bo[None, None, :]
    return out, res


def kernel(**inputs):
    out, _ = run(inputs)
    return out


# revision 9
# speedup vs baseline: 2.1373x; 2.1373x over previous
"""Multi-head causal attention (B=2, S=2048, E=1024, H=16, D=64) on 8 TRN2
NeuronCores.

Sharding (data + tensor parallel, Megatron-style):
  core c -> batch b = c // 4, head group g = c % 4 (4 heads, e' = 256 cols).
  Wq/Wk/Wv column-sharded ([256, 1024] slices), Wo row-sharded
  ([1024, 256] slice); each core produces a partial output [2048, 1024]
  (f16) which the host sums per batch group (the Megatron all-reduce) and
  adds bo.

Per-core device kernel (matmul operands fp16, accumulate fp32 in PSUM),
structured so the ACT(exp) stream starts ~15us in and never stalls:
  - x tensors arrive host-pre-blocked as [4, 128, 8*512] so each 512-col
    block is one 128-descriptor DMA; blocks are ordered by first use
    (xk asc, xq desc, xv asc) and the K/Q projections chase the DMAs.
  - attention in S^T orientation per (q-tile 512, head-pair chunk c):
    batches of 8 k-tiles: logits pairs (2 heads row-tiled on the PE at
    partitions 0-63/64-127), exp on ACT (1/8 scale folded), triangular
    trimming (diagonal tiles only compute columns >= dd, one [128,2,128]
    lower-tri mask), attnV (V' ones column -> PSUM row 64 = softmax
    denominator) lagging the exp stream by 2 tiles.
  - leftover K/Q projection tiles, V-projection tiles and the previous
    q-tile's O-projection are woven into the attention stream at batch
    boundaries (all 128-contraction) to use the PE slack under ACT.
  - normalize: accs copied to SBUF (frees PSUM fast), denominator rows
    packed [2,512], reciprocal via bit-trick seed + 2 Newton iterations
    on the idle GpSimd(Pool) engine, partition-broadcast, DVE multiply
    into valsT.
"""
import sys
import os

sys.path.insert(0, "/opt/trn_rl_repo")

import numpy as np
from contextlib import ExitStack

import concourse.bass as bass  # noqa: E402
import concourse.mybir as mybir  # noqa: E402
import concourse.tile as tile  # noqa: E402
from concourse import bacc, bass_utils  # noqa: E402

bass_utils.upload_artifacts = lambda d: f"local:{d}"

B, S, E, H, D = 2, 2048, 1024, 16, 64
NCORES = 8
EL = 256  # e' columns per core (4 heads)
F32 = mybir.dt.float32
F16 = mybir.dt.float16
I32 = mybir.dt.int32
AF = mybir.ActivationFunctionType
OP = mybir.AluOpType
NP16 = np.float16

RECIP_SEED = 0x7EF311C3 + 1  # ~(x) + (C+1) == C - x

_CACHE = {}


def _build():
    nc = bacc.Bacc("TRN2", target_bir_lowering=False, debug=False)

    # x tensors host-pre-blocked: [tb, p, k*512 + m]
    xq_d = nc.dram_tensor("xqB", [4, 128, 8 * 512], F16, kind="ExternalInput")
    xk_d = nc.dram_tensor("xkB", [4, 128, 8 * 512], F16, kind="ExternalInput")
    xv_d = nc.dram_tensor("xvB", [4, 128, 8 * 512], F16, kind="ExternalInput")
    wq_d = nc.dram_tensor("wqT", [E, EL], F16, kind="ExternalInput")
    wk_d = nc.dram_tensor("wkT", [E, EL], F16, kind="ExternalInput")
    wv_d = nc.dram_tensor("wvT", [E, EL], F16, kind="ExternalInput")
    wo_d = nc.dram_tensor("woT", [EL, E], F16, kind="ExternalInput")
    bq_d = nc.dram_tensor("bq", [EL], F32, kind="ExternalInput")
    bk_d = nc.dram_tensor("bk", [EL], F32, kind="ExternalInput")
    bv_d = nc.dram_tensor("bv", [EL], F32, kind="ExternalInput")
    vones_d = nc.dram_tensor("vones", [128, 16, 4, 1], F16, kind="ExternalInput")
    mask_d = nc.dram_tensor("masks", [128, 2, 128], F16, kind="ExternalInput")
    out_d = nc.dram_tensor("out", [S, E], F16, kind="ExternalOutput")

    with tile.TileContext(nc) as tc, ExitStack() as ctx:
        cpool = ctx.enter_context(tc.tile_pool(name="const", bufs=1))
        psp = ctx.enter_context(tc.tile_pool(name="psp", bufs=2, space="PSUM"))
        expp = ctx.enter_context(tc.tile_pool(name="expp", bufs=10))
        opool = ctx.enter_context(tc.tile_pool(name="op", bufs=2))
        smp = ctx.enter_context(tc.tile_pool(name="smp", bufs=2))

        xk = cpool.tile([128, 8, S], F16, tag="xk")
        xq = cpool.tile([128, 8, S], F16, tag="xq")
        xv = cpool.tile([128, 8, S], F16, tag="xv")

        def xblock(x_t, x_d, tb):
            nc.sync.dma_start(
                x_t[:, :, tb * 512:(tb + 1) * 512],
                x_d.ap()[tb].rearrange("p (k m) -> p k m", k=8))

        # ---- DMA order == arrival order (single HWDGE ring): weights for
        # K/Q, xk/xq/xv blocks interleaved by first-use time ----
        wk = cpool.tile([128, 8, EL], F16, tag="wk")
        nc.sync.dma_start(wk[:], wk_d.ap().rearrange("(k p) m -> p k m", p=128))
        bkt = cpool.tile([128, 2], F32, tag="bkt")
        nc.sync.dma_start(bkt[:], bk_d.ap().rearrange("(c p) -> p c", p=128))
        wq = cpool.tile([128, 8, EL], F16, tag="wq")
        nc.sync.dma_start(wq[:], wq_d.ap().rearrange("(k p) m -> p k m", p=128))
        bqt = cpool.tile([128, 2], F32, tag="bqt")
        nc.sync.dma_start(bqt[:], bq_d.ap().rearrange("(c p) -> p c", p=128))
        xblock(xk, xk_d, 0)
        xblock(xk, xk_d, 1)
        xblock(xq, xq_d, 3)
        xblock(xk, xk_d, 2)
        xblock(xk, xk_d, 3)
        xblock(xq, xq_d, 2)
        wv = cpool.tile([128, 8, EL], F16, tag="wv")
        nc.sync.dma_start(wv[:], wv_d.ap().rearrange("(k p) m -> p k m", p=128))
        bvr = cpool.tile([1, EL], F32, tag="bvr")
        nc.sync.dma_start(bvr[:], bv_d.ap().rearrange("(p m) -> p m", p=1))
        bvb = cpool.tile([128, EL], F32, tag="bvb")
        nc.gpsimd.partition_broadcast(bvb[:], bvr[:])
        VP = cpool.tile([128, 16, 4 * 66], F16, tag="VP")  # 66: 4B-aligned
        nc.sync.dma_start(
            VP[:].rearrange("p k (h x) -> p k h x", h=4)[:, :, :, 64:65],
            vones_d.ap())
        xblock(xv, xv_d, 0)
        xblock(xv, xv_d, 1)
        xblock(xq, xq_d, 1)
        xblock(xq, xq_d, 0)
        xblock(xv, xv_d, 2)
        xblock(xv, xv_d, 3)
        mk2 = cpool.tile([128, 2, 128], F16, tag="mk2")
        nc.sync.dma_start(mk2[:], mask_d.ap())
        wo = cpool.tile([128, 2, E], F16, tag="wo")
        nc.sync.dma_start(wo[:], wo_d.ap().rearrange("(c p) m -> p c m", p=128))

        KT = cpool.tile([128, 2, S], F16, tag="KT")
        QT = cpool.tile([128, 2, S], F16, tag="QT")
        valsT = cpool.tile([128, 2, S], F16, tag="valsT")

        def kproj_tb(tb, tag):
            for c in range(2):
                ps = psp.tile([128, 512], F32, tag=tag,
                              bufs=2, name=f"kps{tb}_{c}")
                for k in range(8):
                    nc.tensor.matmul(
                        ps[:],
                        lhsT=wk[:, k, c * 128:(c + 1) * 128],
                        rhs=xk[:, k, tb * 512:(tb + 1) * 512],
                        start=(k == 0), stop=(k == 7))
                nc.vector.tensor_scalar_add(
                    KT[:, c, tb * 512:(tb + 1) * 512], ps[:], bkt[:, c:c + 1])

        def qproj_tt(tt, tag):
            for c in range(2):
                ps = psp.tile([128, 512], F32, tag=tag,
                              bufs=2, name=f"qps{tt}_{c}")
                for k in range(8):
                    nc.tensor.matmul(
                        ps[:],
                        lhsT=wq[:, k, c * 128:(c + 1) * 128],
                        rhs=xq[:, k, tt * 512:(tt + 1) * 512],
                        start=(k == 0), stop=(k == 7))
                nc.vector.tensor_scalar_add(
                    QT[:, c, tt * 512:(tt + 1) * 512], ps[:], bqt[:, c:c + 1])

        # K blocks 0,1 + Q tile 3 up front: first exp fires after these
        kproj_tb(0, "lg")
        kproj_tb(1, "lg")
        qproj_tt(3, "lg")

        def vproj_tile(t3):
            ps = psp.tile([128, EL], F32, tag="ops", bufs=2, name=f"vps{t3}")
            for k in range(8):
                nc.tensor.matmul(
                    ps[:],
                    lhsT=xv[:, k, t3 * 128:(t3 + 1) * 128],
                    rhs=wv[:, k, :],
                    start=(k == 0), stop=(k == 7))
            nc.vector.tensor_add(
                VP[:, t3, :].rearrange("p (h x) -> p h x", h=4)[:, :, 0:64],
                ps[:].rearrange("p (h x) -> p h x", h=4),
                bvb[:].rearrange("p (h x) -> p h x", h=4))

        def oproj_tt(tt):
            ot = opool.tile([128, 2, 512], F16, tag="ot", name=f"ot{tt}")
            for eo in range(2):
                ps = psp.tile([128, 512], F32, tag="ops", bufs=2,
                              name=f"ops{tt}_{eo}")
                for c in range(2):
                    nc.tensor.matmul(
                        ps[:],
                        lhsT=valsT[:, c, tt * 128:(tt + 1) * 128],
                        rhs=wo[:, c, eo * 512:(eo + 1) * 512],
                        start=(c == 0), stop=(c == 1))
                nc.vector.tensor_copy(ot[:, eo, :], ps[:])
            nc.sync.dma_start(
                out_d.ap()[tt * 128:(tt + 1) * 128, :],
                ot[:].rearrange("p a b -> p (a b)"))

        # 128-contraction work woven in at attention batch boundaries
        boundary = {
            (3, 0, 0): lambda: (kproj_tb(2, "ops"), kproj_tb(3, "ops")),
            (3, 0, 8): lambda: qproj_tt(2, "ops"),
            (3, 1, 0): lambda: qproj_tt(1, "ops"),
            (3, 1, 8): lambda: qproj_tt(0, "ops"),
            (2, 0, 0): lambda: (oproj_tt(12), oproj_tt(13)),
            (2, 0, 8): lambda: (oproj_tt(14), oproj_tt(15)),
            (1, 0, 0): lambda: (oproj_tt(8), oproj_tt(9)),
            (1, 1, 0): lambda: (oproj_tt(10), oproj_tt(11)),
            (0, 0, 0): lambda: (oproj_tt(4), oproj_tt(5)),
            (0, 1, 0): lambda: (oproj_tt(6), oproj_tt(7)),
        }

        for qt in range(3, -1, -1):
            nkt = 4 * qt + 4
            accs = {}
            for c in range(2):
                for hh in range(2):
                    accs[(c, hh)] = psp.tile([65, 512], F32, tag="acc",
                                             bufs=2, name=f"acc{qt}_{c}_{hh}")
            exs = {}

            def lg_exp(c, kt):
                dd = kt * 128 - qt * 512
                s = max(dd, 0)
                lg = psp.tile([128, 2, 512], F32, tag="lg", bufs=2,
                              name=f"lg{qt}_{c}_{kt}")
                for hh in range(2):
                    nc.tensor.matmul(
                        lg[:, hh, s:512],
                        lhsT=KT[hh * 64:(hh + 1) * 64, c,
                                kt * 128:(kt + 1) * 128],
                        rhs=QT[hh * 64:(hh + 1) * 64, c,
                               qt * 512 + s:(qt + 1) * 512],
                        start=True, stop=True)
                ex = expp.tile([128, 2, 512], F16, tag="ex",
                               name=f"ex{qt}_{c}_{kt}")
                nc.scalar.activation(ex[:, :, s:512], lg[:, :, s:512], AF.Exp,
                                     scale=0.125)
                if dd >= 0:  # diagonal tile: lower-tri mask on first 128 cols
                    nc.vector.tensor_mul(ex[:, :, s:s + 128],
                                         ex[:, :, s:s + 128], mk2[:])
                exs[(c, kt)] = ex

            def attn_v(c, kt):
                ex = exs.pop((c, kt))
                s = max(kt * 128 - qt * 512, 0)
                for hh in range(2):
                    h = 2 * c + hh
                    nc.tensor.matmul(
                        accs[(c, hh)][:, s:512],
                        lhsT=VP[:, kt, h * 66:h * 66 + 65],
                        rhs=ex[:, hh, s:512],
                        start=(kt == 0), stop=(kt == nkt - 1),
                        skip_group_check=True)

            for c in range(2):
                pend = 0
                for b0 in range(0, nkt, 8):
                    bend = min(b0 + 8, nkt)
                    for kt in range(b0, bend):
                        lg_exp(c, kt)
                    bw = boundary.get((qt, c, b0))
                    if bw is not None:
                        bw()
                    while pend <= bend - 3:
                        if qt == 3 and c == 0:
                            vproj_tile(pend)
                        attn_v(c, pend)
                        pend += 1
                while pend < nkt:
                    if qt == 3 and c == 0:
                        vproj_tile(pend)
                    attn_v(c, pend)
                    pend += 1

                # ---- normalize chunk c ----
                # denominator rows (PSUM row 64) -> srow2 rows 0/1; acc body
                # -> SBUF sv (frees the PSUM slot); reciprocal via bit-trick
                # seed + 2 Newton iterations on Pool; broadcast; multiply.
                svs = {}
                for hh in range(2):
                    sv = smp.tile([65, 512], F32, tag="sv", bufs=4,
                                  name=f"sv{qt}_{c}_{hh}")
                    nc.vector.tensor_copy(sv[:], accs[(c, hh)][:])
                    svs[hh] = sv
                # denominator rows DMA-transposed to [128, 4] per hh so the
                # reciprocal runs partition-parallel (4 elems/lane)
                lcol = smp.tile([128, 8], F32, tag="lcol", name=f"lc{qt}_{c}")
                for hh in range(2):
                    nc.sync.dma_start(
                        lcol[:, hh * 4:(hh + 1) * 4],
                        svs[hh][64:65, :].rearrange("p (a b) -> p a b", a=128))
                rcol = smp.tile([128, 8], F32, tag="rcol", name=f"rc{qt}_{c}")
                nc.vector.reciprocal(rcol[:], lcol[:])
                rrow = smp.tile([1, 2, 512], F32, tag="rrow", bufs=1,
                                name=f"rr{qt}_{c}")
                for hh in range(2):
                    nc.sync.dma_start(
                        rrow[0:1, hh, :].rearrange("p (a b) -> p a b", a=128),
                        rcol[:, hh * 4:(hh + 1) * 4])
                for hh in range(2):
                    bc = smp.tile([128, 512], F32, tag="bc", bufs=4,
                                  name=f"bc{qt}_{c}_{hh}")
                    nc.gpsimd.partition_broadcast(bc[0:64, :], rrow[0:1, hh, :])
                    nc.vector.tensor_mul(
                        valsT[hh * 64:(hh + 1) * 64, c,
                              qt * 512:(qt + 1) * 512],
                        svs[hh][0:64, :], bc[0:64, :])

        # tail: first q-tile's O-projection
        for tt in range(4):
            oproj_tt(tt)

    nc.compile()
    return nc


def get_nc():
    if "nc" not in _CACHE:
        _CACHE["nc"] = _build()
    return _CACHE["nc"]


def _masks():
    i = np.arange(128)[:, None]
    j = np.arange(128)[None, :]
    m = (i <= j).astype(NP16)  # within-window causal: keep k <= q
    return np.broadcast_to(m[:, None, :], (128, 2, 128)).copy()


def _xblocks(x):
    # [S, E] f32 -> [4, 128, 8*512] f16: blk[tb, p, k*512+m] = x[tb*512+m, k*128+p]
    xT = np.ascontiguousarray(x.T).astype(NP16)  # [E, S]
    return np.ascontiguousarray(
        xT.reshape(8, 128, 4, 512).transpose(2, 1, 0, 3).reshape(4, 128, 4096))


def make_in_maps(query, key, value, Wq, bq, Wk, bk, Wv, bv, Wo, bo):
    query = np.asarray(query, np.float32)
    key = np.asarray(key, np.float32)
    value = np.asarray(value, np.float32)
    Wq, Wk, Wv, Wo = (np.asarray(a, np.float32) for a in (Wq, Wk, Wv, Wo))
    bq, bk, bv = (np.asarray(a, np.float32) for a in (bq, bk, bv))
    masks = _masks()
    vones = np.ones((128, 16, 4, 1), NP16)
    xb = {}
    for b in range(B):
        xb[b] = (_xblocks(query[b]), _xblocks(key[b]), _xblocks(value[b]))
    in_maps = []
    for c in range(NCORES):
        b, g = divmod(c, 4)
        sl = slice(g * EL, (g + 1) * EL)
        in_maps.append({
            "xqB": xb[b][0],
            "xkB": xb[b][1],
            "xvB": xb[b][2],
            "wqT": np.ascontiguousarray(Wq[sl, :].T).astype(NP16),
            "wkT": np.ascontiguousarray(Wk[sl, :].T).astype(NP16),
            "wvT": np.ascontiguousarray(Wv[sl, :].T).astype(NP16),
            "woT": np.ascontiguousarray(Wo[:, sl].T).astype(NP16),
            "bq": np.ascontiguousarray(bq[sl]),
            "bk": np.ascontiguousarray(bk[sl]),
            "bv": np.ascontiguousarray(bv[sl]),
            "vones": vones,
            "masks": masks,
        })
    return in_maps


def run(inputs, trace=False, tmpdir=None):
    """Run on 8 cores; returns (full_output, BassKernelResults)."""
    nc = get_nc()
    in_maps = make_in_maps(**inputs)
    res = bass_utils.run_bass_kernel_spmd(
        nc, in_maps, list(range(NCORES)), trace=trace, tmpdir=tmpdir)
    bo = np.asarray(inputs["bo"], np.float32)
    out = np.zeros((B, S, E), np.float32)
    for c in range(NCORES):
        out[c // 4] += res.results[c]["out"]
    out += bo[None, None, :]
    return out, res


def kernel(**inputs):
    out, _ = run(inputs)
    return out
